# revision 14
# baseline (speedup 1.0000x reference)
"""Dynamic-conv (CondConv-style) kernel for Trainium2, 8 NeuronCores.

Problem: for each sample b:
    se     = global-avg-pool(x[b])                     (256,)
    gates  = sigmoid(se @ route_w.T + route_b)         (8,)
    w_dyn  = (gates @ weight.T).reshape(256,256,3,3)   per-sample 3x3 conv kernel
    out[b] = conv2d(x[b], w_dyn, pad=1) + bias         (256,28,28)

Sharding: data-parallel over batch, 4 samples per core; the expert weight
bank and routing weights are replicated.

Per-core plan (v2 — latency-packed schedule):
  - x arrives host-padded (30x30 bf16, channel-permuted): one clean DMA per
    (sample, ci-half), no on-device pad.  Pooling is a single DVE
    tensor_reduce per tile (pad zeros don't change the sum).
  - gates: tiny bf16 matmul rwx^T @ se (rhs [128,4]) -> sigmoid ->
    4 masked per-sample broadcasts build the block-diagonal stationary
    G[(q,e),(s,q')] = gate[s,e] * (q==q').
  - kernel synthesis on PE in 6 f-window chunks S0..S5 paced by the bank
    DMA (6x 1.57MB).  Each chunk: 8 clp x 2 col-tiled matmuls (128-wide
    contraction), PSUM drained (cast bf16) alternately on ACT/DVE into a
    (128, 8, 3, 384) stage; SBUF->SBUF DMAs re-gather w_dyn with ci on
    partitions (u=0 on sync/HWDGE, u=1 on gpsimd/SWDGE).
  - conv: 9 shifted bf16 matmuls accumulating over (ci_tile, kh, kw) in
    PSUM; ACT adds bias on the drain; fp32 DMA out on the ACT queue.
  - PE program order interleaves synthesis chunks with conv blocks so the
    PE never queues behind a re-gather:  S0 S1 S2, C(h0,s0), S3, C(h0,s1),
    S4, C(h0,s2), S5, C(h0,s3), C(h1,s0..s3).
  - warmup/filler matmuls on garbage data cover the preamble and the
    bank-paced gaps so the PE HAM clock-gate stays at full rate.
"""

import os
from contextlib import ExitStack

import ml_dtypes
import numpy as np

import concourse.bacc as bacc
import concourse.bass as bass
import concourse.mybir as mybir
import concourse.tile as tile
from concourse.bass_utils import run_bass_kernel_spmd

FP32 = mybir.dt.float32
BF16 = mybir.dt.bfloat16
BF16_NP = ml_dtypes.bfloat16

N_CORES = 8
B, C_IN, H, W = 32, 256, 28, 28
NUM, C_OUT, K = 8, 256, 3
BS = B // N_CORES          # samples per core = 4
NQ = 16                    # ci chunks in the synthesis contraction
F = 2304                   # f = co_t*1152 + khkw*128 + co_lo
NWIN = 384                 # synthesis matmul free size
HP = H + 2                 # host-padded spatial size

N_WARM = 36                # warmup matmuls covering the preamble
N_FILL = (14, 14, 18)      # fillers after S0 / S1 / S2 (bank-paced gaps)


def build_nc() -> bacc.Bacc:
    nc = bacc.Bacc("TRN2", target_bir_lowering=False, debug=False,
                   num_devices=N_CORES)

    # host-padded, channel-permuted, partition-major bf16 input
    x_d = nc.dram_tensor("x", [BS, 128, 2, HP, HP], BF16, kind="ExternalInput")
    # bank[p=(q,e), w, cl, n] = W[e, ci=q*16+cl, f=w*384+n]; per-partition
    # contiguous so one DMA per w moves 12.3KB/partition runs.
    bank_d = nc.dram_tensor("bank", [128, 6, NQ, NWIN], BF16, kind="ExternalInput")
    rwx_d = nc.dram_tensor("rwx", [128, 2, 128], BF16, kind="ExternalInput")
    rbx_d = nc.dram_tensor("rbx", [128, 1], FP32, kind="ExternalInput")
    mask_d = nc.dram_tensor("mask16", [128, NQ], BF16, kind="ExternalInput")
    bias_d = nc.dram_tensor("bias", [128, 2], FP32, kind="ExternalInput")
    out_d = nc.dram_tensor("out", [BS, C_OUT, H, W], FP32, kind="ExternalOutput")
    dbg = os.environ.get("KERNEL_DEBUG_TAPS")
    if dbg:
        dbgG_d = nc.dram_tensor("dbgG", [128, 64], BF16, kind="ExternalOutput")
        dbgS_d = nc.dram_tensor("dbgS", [128, 8, 3, 384], BF16, kind="ExternalOutput")
        dbgW_d = nc.dram_tensor("dbgW", [128, 9, 128], BF16, kind="ExternalOutput")

    with tile.TileContext(nc) as tc, ExitStack() as ctx:
        singles = ctx.enter_context(tc.tile_pool(name="singles", bufs=1))
        xpadp = ctx.enter_context(tc.tile_pool(name="xpadp", bufs=1))
        bankp = ctx.enter_context(tc.tile_pool(name="bankp", bufs=3))
        bankp2 = ctx.enter_context(tc.tile_pool(name="bankp2", bufs=1))
        stagep = ctx.enter_context(tc.tile_pool(name="stagep", bufs=2))
        wdynp = ctx.enter_context(tc.tile_pool(name="wdynp", bufs=1))
        outp = ctx.enter_context(tc.tile_pool(name="outp", bufs=2))
        psS = ctx.enter_context(tc.tile_pool(name="psS", bufs=4, space="PSUM"))
        psC = ctx.enter_context(tc.tile_pool(name="psC", bufs=4, space="PSUM"))

        # ---- replicated constants (gpsimd/SWDGE queue: keeps the HWDGE
        # semaphore pool free for x + bank + re-gathers)
        rwx = singles.tile([128, 2, 128], BF16)
        nc.gpsimd.dma_start(out=rwx, in_=rwx_d[:])
        rbx = singles.tile([128, 1], FP32)
        nc.gpsimd.dma_start(out=rbx, in_=rbx_d[:])
        mask16 = singles.tile([128, NQ], BF16)
        nc.gpsimd.dma_start(out=mask16, in_=mask_d[:])
        biasT = singles.tile([128, 2], FP32)
        nc.gpsimd.dma_start(out=biasT, in_=bias_d[:])
        garbage = singles.tile([128, NWIN], BF16)
        nc.vector.memset(garbage, 1.0)
        warm = singles.tile([128, 1], FP32)
        nc.scalar.activation(out=warm, in_=garbage[:, 0:1],
                             func=mybir.ActivationFunctionType.Sigmoid)

        # ---- x in (sync queue, first so it owns the HBM stream)
        xtile = {}
        for s in range(BS):
            xs = xpadp.tile([128, 2, HP, HP], BF16, tag=f"xp{s}")
            nc.sync.dma_start(out=xs, in_=x_d[s])
            xtile[s] = xs

        # ---- bank prefetch (sync queue, behind x): w0..w2 singly so the
        # early synthesis chunks are paced per-window; w3..5 as one DMA
        bkt = {}
        for w in range(3):
            bk = bankp.tile([128, NQ, NWIN], BF16, tag="bk", name=f"bk{w}")
            nc.sync.dma_start(out=bk, in_=bank_d[:, w])
            bkt[w] = bk
        bk345 = bankp2.tile([128, 3, NQ, NWIN], BF16, tag="bk345")
        nc.sync.dma_start(out=bk345, in_=bank_d[:, 3:6])

        # ---- PE warmup: keep the HAM clock-gate hot through the preamble
        wps = psC.tile([128, NWIN], FP32, tag="pc", name="warmps")
        for i in range(N_WARM):
            nc.tensor.matmul(wps, lhsT=garbage[:, 0:128], rhs=garbage,
                             start=True, stop=True)

        # ---- pooling: one reduce per tile (host pad zeros are no-ops)
        se = singles.tile([128, 2, BS], FP32)
        for s in range(BS):
            for t in range(2):
                nc.vector.tensor_reduce(
                    out=se[:, t, s:s + 1], in_=xtile[s][:, t],
                    axis=mybir.AxisListType.XY, op=mybir.AluOpType.add)
        seb = singles.tile([128, 2, BS], BF16)
        nc.vector.tensor_copy(out=seb, in_=se)

        # ---- gates -> block-diagonal stationary G (M is (s, q) sample-major)
        L = psS.tile([128, BS], FP32, tag="ps", name="Lpsum")
        for t in range(2):
            nc.tensor.matmul(L, lhsT=rwx[:, t, :], rhs=seb[:, t, :],
                             start=(t == 0), stop=(t == 1))
        g0 = singles.tile([128, BS], FP32)
        nc.scalar.activation(out=g0, in_=L,
                             func=mybir.ActivationFunctionType.Sigmoid,
                             bias=rbx, scale=1.0)
        G = singles.tile([128, BS, NQ], BF16)
        for s in range(BS):
            nc.vector.tensor_scalar(
                out=G[:, s, :], in0=mask16,
                scalar1=g0[:, s:s + 1], scalar2=None,
                op0=mybir.AluOpType.mult)
        Gm = G[:, :, :]  # [128, 64] stationary, col m = 16*s + (8*t + q')
        if dbg:
            nc.sync.dma_start(out=dbgG_d[:], in_=Gm)

        # ---- synthesis chunk: one f-window w on the PE
        stage = {}

        def synth(w):
            half, wloc = divmod(w, 3)
            if wloc == 0:
                stage[half] = stagep.tile([128, 8, 3, NWIN], BF16, tag="stage",
                                          name=f"stage{half}")
            st = stage[half]
            for clp in range(8):
                # u=0 and u=1 share one (128,384) PSUM tile via column-strip
                # offsets (col-tiled, concurrent); one drain covers both
                ps = psS.tile([128, NWIN], FP32, tag="ps",
                              name=f"ps{w}_{clp}")
                for u in range(2):
                    rhs = (bkt[w][:, 8 * u + clp, :] if w < 3
                           else bk345[:, w - 3, 8 * u + clp, :])
                    nc.tensor.matmul(ps[64 * u:64 * (u + 1), :], lhsT=Gm,
                                     rhs=rhs, start=True, stop=True)
                dst = st[:, clp, wloc, :]
                if clp % 2 == 0:
                    nc.scalar.activation(
                        out=dst, in_=ps,
                        func=mybir.ActivationFunctionType.Copy)
                else:
                    nc.vector.tensor_copy(out=dst, in_=ps)

        # ---- re-gather one half's w_dyn into conv-stationary layout
        wd = {}

        def regather(half):
            st = stage[half]
            if dbg and half == 0:
                nc.sync.dma_start(out=dbgS_d[:], in_=st)
            for s in range(BS):
                for t in range(2):
                    wdt = wdynp.tile([128, 9, 128], BF16, tag=f"wd{half}{s}{t}")
                    # stage partition 64u+16s+8t+q' holds ci=(8t+q')*16+clp+8u
                    # for clp in the free dim; with the host-side ci
                    # permutation, partition d=64u+8q'+clp of the conv
                    # stationary IS that channel, so both dst slices are
                    # contiguous.  u=0 on sync/HWDGE, u=1 on gpsimd/SWDGE.
                    for u in range(2):
                        src = st[64 * u + NQ * s + 8 * t:
                                 64 * u + NQ * s + 8 * t + 8]
                        eng = nc.sync if u == 0 else nc.gpsimd
                        eng.dma_start(out=wdt[64 * u:64 * (u + 1)], in_=src)
                    wd[half, s, t] = wdt
                    if dbg and half == 0 and s == 0 and t == 0:
                        nc.sync.dma_start(out=dbgW_d[:], in_=wdt)

        # ---- conv block for one (half, sample)
        def conv(half, s):
            pst = [psC.tile([128, 14, W], FP32, tag="pc",
                            name=f"pc{half}_{s}_{c}") for c in range(2)]
            for t in range(2):
                for k in range(9):
                    kh, kw = divmod(k, 3)
                    lw = wd[half, s, t][:, k, :]
                    for c in range(2):
                        rhs = xtile[s][:, t, c * 14 + kh:c * 14 + kh + 14,
                                       kw:kw + W]
                        nc.tensor.matmul(
                            pst[c], lhsT=lw, rhs=rhs,
                            start=(t == 0 and k == 0),
                            stop=(t == 1 and k == 8),
                        )
            ot = outp.tile([128, 2, 14, W], FP32, tag="ot",
                           name=f"ot{half}_{s}")
            for c in range(2):
                nc.scalar.activation(
                    out=ot[:, c], in_=pst[c],
                    func=mybir.ActivationFunctionType.Identity,
                    bias=biasT[:, half:half + 1], scale=1.0)
            nc.scalar.dma_start(
                out=out_d[s, half * 128:(half + 1) * 128], in_=ot)

        def filler(n, name):
            fps = psC.tile([128, NWIN], FP32, tag="pc", name=name)
            for i in range(n):
                nc.tensor.matmul(fps, lhsT=garbage[:, 0:128], rhs=garbage,
                                 start=True, stop=True)

        # ---- packed PE schedule
        synth(0)
        filler(N_FILL[0], "fill0")
        synth(1)
        filler(N_FILL[1], "fill1")
        synth(2)
        filler(N_FILL[2], "fill2")
        regather(0)
        conv(0, 0)
        synth(3)
        conv(0, 1)
        synth(4)
        conv(0, 2)
        synth(5)
        conv(0, 3)
        regather(1)
        for s in range(BS):
            conv(1, s)
    nc.finalize()
    return nc


# partition d (within a 128-channel tile) holds channel perm[d]:
# d = 64u + 8q' + clp  <->  ci_lo = 16q' + 8u + clp
CI_PERM = np.array([(d % 64) // 8 * 16 + (d // 64) * 8 + d % 8
                    for d in range(128)])
CI_MAP = np.concatenate([CI_PERM, 128 + CI_PERM])


def _host_prep(route_w, route_b, weight, bias):
    """Host-side layout transforms (pure numpy, replicated to every core)."""
    We = np.ascontiguousarray(weight.T).reshape(NUM, C_OUT, C_IN, K, K)
    Wf = We.transpose(0, 2, 1, 3, 4)            # [e, ci, co, kh, kw]
    Wf = Wf.reshape(NUM, C_IN, 2, 128, 9)       # [e, ci, co_t, co_lo, khkw]
    Wf = Wf.transpose(0, 1, 2, 4, 3)            # [e, ci, co_t, khkw, co_lo]
    Wf = Wf.reshape(NUM, C_IN, F)               # f = co_t*1152 + khkw*128 + co_lo
    Bk = Wf.reshape(NUM, NQ, NQ, 6, NWIN)       # [e, q, cl, w, n]
    bank = np.ascontiguousarray(
        Bk.transpose(1, 0, 3, 2, 4).reshape(128, 6, NQ, NWIN)).astype(BF16_NP)

    rwx = np.tile((route_w / (H * W)).T, (1, NQ))[CI_MAP]
    rwx = np.ascontiguousarray(
        rwx.reshape(2, 128, 128).transpose(1, 0, 2)).astype(BF16_NP)
    rbx = np.tile(route_b, NQ).reshape(128, 1).astype(np.float32)
    # G column m = (s, q): q(m) = m % 16; mask16[p, q] = (p//8 == q)
    mask16 = (np.arange(128)[:, None] // 8 == np.arange(NQ)[None, :]
              ).astype(BF16_NP)
    bias2 = np.ascontiguousarray(bias.reshape(2, 128).T).astype(np.float32)
    return bank, rwx, rbx, mask16, bias2


def _ensure_ntff_hook():
    """Provide antenv.axon_hooks (absent in this image) so trace=True works.

    The boot script ships a ctypes NTFF hook but can only register it through
    antenv.axon_hooks; shim that module and register the hook ourselves.
    """
    import sys
    import types
    try:
        from antenv.axon_hooks import get_axon_ntff_profile_hook  # noqa: F401
        return
    except ImportError:
        pass
    try:
        import antenv
        from trn_agent_boot.trn_boot import _ntff_profile_via_ctypes
    except ImportError:
        return
    mod = types.ModuleType("antenv.axon_hooks")
    holder = {"hook": None}
    mod.set_axon_ntff_profile_hook = lambda h: holder.__setitem__("hook", h)
    mod.get_axon_ntff_profile_hook = lambda: holder["hook"]
    sys.modules["antenv.axon_hooks"] = mod
    antenv.axon_hooks = mod
    mod.set_axon_ntff_profile_hook(
        _ntff_profile_via_ctypes("/opt/axon/libaxon_pjrt.so"))


_NC_CACHE = None


def kernel(inputs, route_w, route_b, weight, bias):
    global _NC_CACHE
    inputs = np.asarray(inputs, dtype=np.float32)
    route_w = np.asarray(route_w, dtype=np.float32)
    route_b = np.asarray(route_b, dtype=np.float32)
    weight = np.asarray(weight, dtype=np.float32)
    bias = np.asarray(bias, dtype=np.float32)

    bank, rwx, rbx, mask16, bias2 = _host_prep(route_w, route_b, weight, bias)

    if _NC_CACHE is None:
        _NC_CACHE = build_nc()
    nc = _NC_CACHE

    shared = {"bank": bank, "rwx": rwx, "rbx": rbx, "mask16": mask16,
              "bias": bias2}
    xpad = np.zeros((B, 128, 2, HP, HP), dtype=BF16_NP)
    xpad[:, :, :, 1:H + 1, 1:W + 1] = (
        inputs[:, CI_MAP].reshape(B, 2, 128, H, W)
        .transpose(0, 2, 1, 3, 4).astype(BF16_NP))
    in_maps = [
        {"x": np.ascontiguousarray(xpad[BS * c:BS * (c + 1)]), **shared}
        for c in range(N_CORES)
    ]
    trace = bool(int(os.environ.get("KERNEL_TRACE", "0")))
    if trace:
        _ensure_ntff_hook()
    res = run_bass_kernel_spmd(
        nc, in_maps, core_ids=list(range(N_CORES)), trace=trace,
        tmpdir=os.environ.get("KERNEL_TMPDIR"),
    )
    out = np.concatenate([res.results[c]["out"] for c in range(N_CORES)], axis=0)
    kernel.last_results = res
    return out


kernel.last_results = None
